# revision 1
# baseline (speedup 1.0000x reference)
"""GNN message passing (scatter-add of gathered node features) on 8 TRN2 NeuronCores.

Strategy (edge + node hybrid sharding, no collectives):
  - Outputs are node-sharded: core k owns destination rows [k*12500, (k+1)*12500).
  - Edges are assigned to the core owning their destination row.
  - Per core, each edge is one "token": gather x[col] (one 256B row) from HBM via
    dma_gather into an SBUF message buffer, then accumulate into the core's output
    shard in HBM via dma_scatter_add (SDMA CCE read-modify-write add descriptors).
  - dma_gather indices are int16, so x is addressed in 4 segments of 25000 rows;
    tokens inside each block are grouped by source segment (<=4 sub-gathers/block).
  - Duplicate-destination correctness: concurrent CCE RMW descriptors to the same
    row race (hardware-verified), and per-engine ring order does NOT serialize the
    read-modify-write. Therefore destination rows are UNIQUE within each scatter
    call (each row's edges are dealt to distinct blocks on the host) and scatter
    calls are serialized by waiting each scatter's completion semaphore before
    issuing the next. Rows with more edges than there are main blocks spill into
    extra cleanup blocks. Gathers run pipelined ahead on a separate SWDGE queue.
"""

import numpy as np

# ---- problem constants (hardcoded; must match the harness inputs) ----
N_NODES = 100000
N_EDGES = 1250000
D = 64
NCORES = 8

DEFAULT_PARAMS = dict(
    n_nodes=N_NODES,
    d=D,
    ncores=8,
    shard=12500,      # destination rows per core  (ncores*shard == n_nodes)
    nseg=4,           # x segments for int16 gather indices
    nblk=52,          # main (unique-destination) blocks (~24 chunks each;
                      # >~40-chunk blocks overflow the SWDGE ring and hang)
    nbuf=3,           # message buffers in flight
)


def host_prep(x, edge_index, params=DEFAULT_PARAMS):
    """Deal each destination row's edges across distinct blocks (uniqueness
    within a block), group by source segment within a block, pad to 128-token
    chunks. All cores share one program: per-(block, seg) chunk counts are
    maxed over cores. Returns (per_core_inputs, T, blocks, out_rows, trash)."""
    p = params
    ncores, shard, nseg, nblk = p["ncores"], p["shard"], p["nseg"], p["nblk"]
    segsz = p["n_nodes"] // nseg
    assert nseg * segsz == p["n_nodes"] and ncores * shard == p["n_nodes"]
    trash = shard + (-shard) % 128
    out_rows = trash + 128

    row = np.asarray(edge_index[0], dtype=np.int64)
    col = np.asarray(edge_index[1], dtype=np.int64)

    # ---- per-core edge lists with block assignment ----
    core_of = row // shard
    per_core_edges = []   # (blk, seg, c_loc, r_loc) arrays
    max_k = 0
    for k in range(ncores):
        m = core_of == k
        r = (row[m] - k * shard).astype(np.int64)
        c = col[m]
        order = np.argsort(r, kind="stable")
        r, c = r[order], c[order]
        # rank of each edge within its row group: 0..k_r-1
        grp_start = np.r_[0, np.nonzero(np.diff(r))[0] + 1]
        counts = np.diff(np.r_[grp_start, len(r)])
        max_k = max(max_k, int(counts.max()) if len(counts) else 0)
        rank = np.arange(len(r)) - np.repeat(grp_start, counts)
        # pseudo-random per-row start offset for balance
        h = (r * 2654435761) % nblk
        blk = (np.repeat(h[grp_start], counts) + rank)  # rank < nblk -> main
        seg = c // segsz
        per_core_edges.append((blk, rank, seg,
                               (c - seg * segsz).astype(np.int16),
                               r.astype(np.int16)))

    n_clean = max(2, max_k - nblk)   # cleanup blocks for spilled ranks
    nblk_tot = nblk + n_clean

    # resolve final block id (main: (h+rank) % nblk ; spill: nblk + (rank-nblk))
    counts_bs = np.zeros((ncores, nblk_tot, nseg), dtype=np.int64)
    resolved = []
    for k in range(ncores):
        blk, rank, seg, c_loc, r_loc = per_core_edges[k]
        main = rank < nblk
        b = np.where(main, blk % nblk, nblk + (rank - nblk))
        assert b.max(initial=0) < nblk_tot
        np.add.at(counts_bs[k], (b, seg), 1)
        resolved.append((b, seg, c_loc, r_loc))

    # per-(block, seg) chunk counts, shared across cores
    chunks_bs = -(np.max(counts_bs, axis=0) // -128)   # [nblk_tot, nseg]
    tok_bs = chunks_bs * 128
    # token offset of each (block, seg) group in the global stream
    off_bs = np.zeros_like(tok_bs)
    off = 0
    blocks = []   # per block: (tok0, ntok, [(seg, sub_tok0, nchunks), ...])
    for b in range(nblk_tot):
        tok0 = off
        subs = []
        for s in range(nseg):
            off_bs[b, s] = off
            if chunks_bs[b, s] > 0:
                subs.append((s, off, int(chunks_bs[b, s])))
            off += int(tok_bs[b, s])
        ntok = off - tok0
        if ntok > 0:
            blocks.append((tok0, ntok, subs))
    T = off
    assert T % 128 == 0

    per_core = []
    x = np.asarray(x, dtype=np.float32)
    for k in range(ncores):
        b, seg, c_loc, r_loc = resolved[k]
        gidx = np.zeros(T, dtype=np.int16)          # pad gathers read x_seg[0]
        sidx = np.full(T, trash, dtype=np.int16)    # pad scatters hit trash row
        # position within each (b, seg) cell
        order = np.lexsort((seg, b))
        bs_sorted = b[order] * nseg + seg[order]
        starts = np.r_[0, np.nonzero(np.diff(bs_sorted))[0] + 1]
        cnts = np.diff(np.r_[starts, len(bs_sorted)])
        within = np.arange(len(bs_sorted)) - np.repeat(starts, cnts)
        tok = off_bs[b[order], seg[order]] + within
        gidx[tok] = c_loc[order]
        sidx[tok] = r_loc[order]
        gw = np.tile(gidx.reshape(-1, 16).T, (8, 1)).copy()
        sw = np.tile(sidx.reshape(-1, 16).T, (8, 1)).copy()
        per_core.append({"x": x, "gidx": gw, "sidx": sw})

    return per_core, T, blocks, out_rows, trash


def build_bass(T, blocks, params=DEFAULT_PARAMS, out_rows=None):
    import concourse.bacc as bacc
    import concourse.mybir as mybir
    import contextlib

    p = params
    d, nseg, nbuf = p["d"], p["nseg"], p["nbuf"]
    segsz = p["n_nodes"] // nseg

    nc = bacc.Bacc(
        None, target_bir_lowering=False, debug=False, num_swdge_queues=2
    )
    x = nc.dram_tensor("x", [p["n_nodes"], d], mybir.dt.float32, kind="ExternalInput")
    gidx = nc.dram_tensor("gidx", [128, T // 16], mybir.dt.int16, kind="ExternalInput")
    sidx = nc.dram_tensor("sidx", [128, T // 16], mybir.dt.int16, kind="ExternalInput")
    out = nc.dram_tensor("out", [out_rows, d], mybir.dt.float32, kind="ExternalOutput")

    NB = len(blocks)
    max_chunks = max(ntok for _, ntok, _ in blocks) // 128
    # cap tokens per DMA call so its descriptor stream fits the SWDGE ring
    # (~256 descs per engine lane; scatter tx pushes ~ntok/8 per lane)
    cap_ch = 15
    # cumulative sub-gather count per buffer slot, for exact gsem waits
    gcnt = [0] * nbuf
    scnt = [0]  # cumulative scatter call count

    with (
        nc.sbuf_tensor([128, T // 16], mybir.dt.int16) as gi_sb,
        nc.sbuf_tensor([128, T // 16], mybir.dt.int16) as si_sb,
        nc.sbuf_tensor([128, nbuf * max_chunks * d], mybir.dt.float32) as msg,
        nc.semaphore("lsem") as lsem,
        nc.semaphore("ssem") as ssem,
        contextlib.ExitStack() as stack,
        nc.Block() as block,
    ):
        gsems = [stack.enter_context(nc.semaphore(f"gsem{i}")) for i in range(nbuf)]

        @block.gpsimd
        def _(g):
            g.dma_start(out=gi_sb[:], in_=gidx[:]).then_inc(lsem, 16)
            g.dma_start(out=si_sb[:], in_=sidx[:]).then_inc(lsem, 16)
            g.wait_ge(lsem, 32)

            def gathers(j):
                tok0, ntok, subs = blocks[j]
                i = j % nbuf
                base = i * max_chunks * d
                for s, sub0, nch in subs:
                    for c0 in range(0, nch, cap_ch):
                        cc = min(cap_ch, nch - c0)
                        p0 = sub0 + c0 * 128
                        boff = base + ((sub0 - tok0) // 128 + c0) * d
                        buf = msg[:, boff:boff + cc * d]
                        g.dma_gather(
                            out_ap=buf.rearrange("p (k dd) -> p k dd", dd=d),
                            in_ap=x[s * segsz:(s + 1) * segsz, :],
                            idxs_ap=gi_sb[:, p0 // 16:(p0 + cc * 128) // 16],
                            num_idxs=cc * 128,
                            num_idxs_reg=cc * 128,
                            elem_size=d,
                            queue_num=1,
                        ).then_inc(gsems[i], 16)
                        gcnt[i] += 1

            def scatter(b):
                tok0, ntok, _ = blocks[b]
                i = b % nbuf
                base = i * max_chunks * d
                g.wait_ge(gsems[i], 16 * gcnt[i])   # all sub-gathers of block b
                nch = ntok // 128
                for c0 in range(0, nch, cap_ch):
                    cc = min(cap_ch, nch - c0)
                    p0 = tok0 + c0 * 128
                    g.dma_scatter_add(
                        out_ap=out[:],
                        in_ap=msg[:, base + c0 * d:base + (c0 + cc) * d].rearrange(
                            "p (k dd) -> p k dd", dd=d),
                        idxs_ap=si_sb[:, p0 // 16:(p0 + cc * 128) // 16],
                        num_idxs=cc * 128,
                        num_idxs_reg=cc * 128,
                        elem_size=d,
                        queue_num=0,
                    ).then_inc(ssem, 16)
                    scnt[0] += 1

            for j in range(min(nbuf - 1, NB)):
                gathers(j)
            for b in range(NB):
                # serialize between blocks: all of block b-1's scatters fully
                # landed (also frees the buffer slot gathers b+nbuf-1 reuse)
                if b > 0:
                    g.wait_ge(ssem, 16 * scnt[0])
                jg = b + nbuf - 1
                if jg < NB:
                    gathers(jg)
                scatter(b)
            g.wait_ge(ssem, 16 * scnt[0])

    nc.compile()
    return nc


def run_spmd(nc, per_core, trace=False):
    from concourse.bass_utils import run_bass_kernel_spmd
    return run_bass_kernel_spmd(
        nc, per_core, core_ids=list(range(len(per_core))), trace=trace
    )


def kernel(x, edge_index, _trace=False, _return_results=False):
    x = np.asarray(x, dtype=np.float32)
    params = DEFAULT_PARAMS
    per_core, T, blocks, out_rows, trash = host_prep(x, edge_index, params)
    nc = build_bass(T, blocks, params, out_rows)
    res = run_spmd(nc, per_core, trace=_trace)
    shard = params["shard"]
    out = np.concatenate(
        [res.results[k]["out"][:shard] for k in range(params["ncores"])], axis=0)
    if _return_results:
        return out, res
    return out



# revision 22
# speedup vs baseline: 1.5849x; 1.5849x over previous
"""GNN message passing (gather + segment-sum) on 8 TRN2 NeuronCores.

Strategy (node-sharded output, no collectives, no DMA scatter):
  - Core k owns destination rows [k*12500, (k+1)*12500); its edges are
    sorted by destination and tiled into 128-token tiles.
  - Messages x[col] are fetched with dma_gather (256B rows) into SBUF.
  - The segment-sum is done on the Tensor engine: for each 128-token tile,
    a 0/1 selection matrix S^T (S^T[p,j] = 1 iff token p's local dst row
    == j) is built on the Vector engine (iota vs per-partition compare),
    and out_tile[128x64] += S @ msg accumulates in PSUM across the tiles
    of each 128-row output tile.  Each output row is written exactly once
    (scalar engine copies PSUM -> SBUF, one contiguous DMA per supergroup
    writes SBUF -> DRAM). This removes the scatter-add RMW traffic and
    serialization that dominated the previous version.
  - Gather indices are int16, so x is addressed in 4 segments of 25000
    rows; the token stream is ordered (supergroup of 6 output tiles,
    segment, output tile) so each (supergroup, segment) run is one large
    gather call (SWDGE prep overhead amortized; ring-safe at <=3072
    tokens/call, 2 calls in flight on alternating queues).
"""

import numpy as np

N_NODES = 100000
N_EDGES = 1250000
D = 64
NCORES = 8
SHARD = N_NODES // NCORES          # 12500 destination rows per core
NSEG = 4
SEGSZ = N_NODES // NSEG            # 25000 (int16-addressable gather window)
OT = -(SHARD // -128)              # 98 output tiles per core
SUPER = 6                          # output tiles per supergroup
NSUP = -(OT // -SUPER)             # 17
MAXTOK = 1024                      # max tokens per gather call (hardware
                                   # crashes above this; 2KB/partition limit)
NBUF = 3                           # gather-call buffers in flight
CHT = 16                           # S-matrix build sub-chunk (tiles)
NPSUM = 8                          # psum slots (1 full 2KB bank each;
                                   # start=True zeroes the whole bank)
OUT_ROWS = OT * 128                # 12544 (padded; host slices to 12500)


def build_schedule(counts_max):
    """Static per-core-shared schedule from per-(otile, seg) max counts."""
    n_tiles = -(counts_max // -128)          # [OT, NSEG]
    for o in range(OT):
        if n_tiles[o].sum() == 0:
            n_tiles[o, 0] = 1
    off = np.zeros((OT, NSEG), np.int64)
    tile_otile = []
    calls = []          # (seg, tok0, ntok)
    sup_tok_end = []
    tok = 0
    for g in range(NSUP):
        otiles = range(g * SUPER, min((g + 1) * SUPER, OT))
        for s in range(NSEG):
            run_t0 = tok
            for o in otiles:
                off[o, s] = tok
                nt = int(n_tiles[o, s])
                tile_otile += [o] * nt
                tok += nt * 128
            ntok = tok - run_t0
            t0 = run_t0
            while ntok > 0:
                take = min(ntok, MAXTOK)
                calls.append((s, t0, take))
                t0 += take
                ntok -= take
        sup_tok_end.append(tok)
    T = tok
    tile_otile = np.asarray(tile_otile)
    TT = T // 128
    first = np.zeros(TT, bool)
    last = np.zeros(TT, bool)
    M_o = np.zeros(OT, np.int64)     # matmul count through otile o's stop
    for o in range(OT):
        idxs = np.nonzero(tile_otile == o)[0]
        first[idxs[0]] = True
        last[idxs[-1]] = True
        M_o[o] = idxs[-1] + 1
    # per-tile call index and in-call tile offset
    call_of_tile = np.zeros(TT, np.int64)
    tile_in_call = np.zeros(TT, np.int64)
    for ci, (s, t0, ntok) in enumerate(calls):
        lo, hi = t0 // 128, (t0 + ntok) // 128
        call_of_tile[lo:hi] = ci
        tile_in_call[lo:hi] = np.arange(hi - lo)
    # tiles completed through call ci (for buffer-reuse pacing)
    tiles_through_call = np.asarray([(t0 + ntok) // 128 for _, t0, ntok in calls])
    return dict(
        n_tiles=n_tiles, off=off, T=T, TT=TT, calls=calls,
        tile_otile=tile_otile, first=first, last=last, M_o=M_o,
        call_of_tile=call_of_tile, tile_in_call=tile_in_call,
        tiles_through_call=tiles_through_call, sup_tok_end=sup_tok_end,
    )


def host_prep(x, edge_index):
    row = np.asarray(edge_index[0]).astype(np.int64)
    col = np.asarray(edge_index[1]).astype(np.int64)
    core = row // SHARD
    per_core_raw = []
    counts = np.zeros((NCORES, OT, NSEG), np.int64)
    for k in range(NCORES):
        m = core == k
        r = row[m] - k * SHARD
        c = col[m]
        o = r >> 7
        lr = r & 127
        s = c // SEGSZ
        cl = (c - s * SEGSZ).astype(np.int16)
        np.add.at(counts[k], (o, s), 1)
        per_core_raw.append((o, s, cl, lr))
    sched = build_schedule(counts.max(axis=0))
    T, off = sched["T"], sched["off"]

    x = np.ascontiguousarray(np.asarray(x, dtype=np.float32))
    per_core = []
    iota = np.broadcast_to(
        np.arange(128, dtype=np.float32), (128, 128))
    for k in range(NCORES):
        o, s, cl, lr = per_core_raw[k]
        key = o * NSEG + s
        order = np.argsort(key, kind="stable")
        ks = key[order]
        starts = np.r_[0, np.nonzero(np.diff(ks))[0] + 1]
        cnts = np.diff(np.r_[starts, len(ks)])
        rank = np.arange(len(ks)) - np.repeat(starts, cnts)
        tokpos = off[o[order], s[order]] + rank
        gidx = np.zeros(T, np.int16)
        gidx[tokpos] = cl[order]
        lrf = np.full(T, 255.0, np.float32)
        lrf[tokpos] = lr[order]
        gw = np.tile(gidx.reshape(-1, 16).T, (8, 1)).copy()
        lrw = np.concatenate([iota, lrf.reshape(-1, 128).T], axis=1).copy()
        per_core.append({"x": x, "gidx": gw, "lrt": lrw})
    return per_core, sched


def build_bass(sched, stage="full", nq=2, upbar=False):
    import concourse.bacc as bacc
    import concourse.mybir as mybir
    import contextlib
    do_vec = stage in ("B", "C", "full")
    do_pe = stage in ("C", "full")
    do_out = stage == "full"

    T, TT = sched["T"], sched["TT"]
    calls = sched["calls"]
    tile_otile = sched["tile_otile"]
    first, last, M_o = sched["first"], sched["last"], sched["M_o"]
    call_of_tile = sched["call_of_tile"]
    tile_in_call = sched["tile_in_call"]
    tiles_through_call = sched["tiles_through_call"]
    sup_tok_end = sched["sup_tok_end"]

    max_call_tiles = max(ntok for _, _, ntok in calls) // 128
    VC = -(TT // -CHT)
    P1 = sup_tok_end[0]          # gidx uploaded in 2 pieces; piece1 covers sup 0
    P1c = P1 // 16

    nc = bacc.Bacc(None, target_bir_lowering=False, debug=False,
                   num_swdge_queues=2)
    x = nc.dram_tensor("x", [N_NODES, D], mybir.dt.float32,
                       kind="ExternalInput")
    gidx = nc.dram_tensor("gidx", [128, T // 16], mybir.dt.int16,
                          kind="ExternalInput")
    lrt = nc.dram_tensor("lrt", [128, 128 + TT], mybir.dt.float32,
                         kind="ExternalInput")
    out = nc.dram_tensor("out", [128, OT * D], mybir.dt.float32,
                         kind="ExternalOutput")

    with (
        nc.sbuf_tensor([128, T // 16], mybir.dt.int16) as gi_sb,
        nc.sbuf_tensor([128, 128 + TT], mybir.dt.float32) as lrt_sb,
        nc.sbuf_tensor([128, NBUF * max_call_tiles * D], mybir.dt.float32) as msg,
        nc.sbuf_tensor([128, 2 * CHT * 128], mybir.dt.float32) as s_sb,
        nc.sbuf_tensor([128, OT * D], mybir.dt.float32) as out_sb,
        nc.psum_tensor([128, NPSUM * 512], mybir.dt.float32) as psum,
        nc.semaphore("lsem_l") as lsem_l,
        nc.semaphore("lsem_a") as lsem_a,
        nc.semaphore("lsem_b") as lsem_b,
        nc.semaphore("gsemA") as gsemA,
        nc.semaphore("gsemB") as gsemB,
        nc.semaphore("vsem") as vsem,
        nc.semaphore("mmsem") as mmsem,
        nc.semaphore("csem") as csem,
        nc.semaphore("wsem") as wsem,
        contextlib.ExitStack() as _stack,
        nc.Block() as block,
    ):

        @block.gpsimd
        def _(g):
            g.dma_start(out=lrt_sb[:], in_=lrt[:]).then_inc(lsem_l, 16)
            g.dma_start(out=gi_sb[:, :P1c], in_=gidx[:, :P1c]).then_inc(lsem_a, 16)
            g.dma_start(out=gi_sb[:, P1c:], in_=gidx[:, P1c:]).then_inc(lsem_b, 16)
            lvl = 0
            if upbar:
                g.wait_ge(lsem_l, 16)
                g.wait_ge(lsem_a, 16)
                g.wait_ge(lsem_b, 16)
                lvl = 2
            if stage == "A0":
                return
            limit = globals().get("LIMIT_CALLS") or len(calls)
            for ci, (s, t0, ntok) in enumerate(calls[:limit]):
                need = 1 if t0 + ntok <= P1 else 2
                if need > lvl:
                    g.wait_ge(lsem_a if need == 1 else lsem_b, 16)
                    lvl = need
                qsem = gsemA if (nq == 1 or ci % 2 == 0) else gsemB
                if ci >= nq:
                    g.wait_ge(qsem, 16 * (ci // nq))
                if do_pe and ci >= NBUF:
                    g.wait_ge(mmsem, int(tiles_through_call[ci - NBUF]))
                base = (ci % NBUF) * max_call_tiles * D
                nt = ntok // 128
                g.dma_gather(
                    out_ap=msg[:, base:base + nt * D].rearrange(
                        "p (k dd) -> p k dd", dd=D),
                    in_ap=x[s * SEGSZ:(s + 1) * SEGSZ, :],
                    idxs_ap=gi_sb[:, t0 // 16:(t0 + ntok) // 16],
                    num_idxs=ntok,
                    num_idxs_reg=ntok,
                    elem_size=D,
                    queue_num=(ci % 2) if nq == 2 else 1,
                ).then_inc(qsem, 16)
            if do_out:
                for gi in range(NSUP):
                    o_lo = gi * SUPER
                    o_hi = min(o_lo + SUPER, OT)
                    g.wait_ge(csem, o_hi)
                    g.dma_start(
                        out=out[:, o_lo * D:o_hi * D],
                        in_=out_sb[:, o_lo * D:o_hi * D],
                    ).then_inc(wsem, 16)
                g.wait_ge(wsem, 16 * NSUP)
            else:
                ncalls = limit
                nA = ncalls if nq == 1 else (ncalls + 1) // 2
                g.wait_ge(gsemA, 16 * nA)
                if nq == 2:
                    g.wait_ge(gsemB, 16 * (ncalls - nA))

        @block.vector
        def _(v):
            if not do_vec:
                return
            v.wait_ge(lsem_l, 16)
            for vc in range(VC):
                if do_pe and vc >= 2:
                    v.wait_ge(mmsem, CHT * (vc - 1))
                t_lo = CHT * vc
                t_hi = min(t_lo + CHT, TT)
                for ti in range(t_lo, t_hi):
                    pos = ((vc % 2) * CHT + (ti - t_lo)) * 128
                    inst = v.tensor_scalar(
                        out=s_sb[:, pos:pos + 128],
                        in0=lrt_sb[:, 0:128],
                        scalar1=lrt_sb[:, 128 + ti:129 + ti],
                        scalar2=None,
                        op0=mybir.AluOpType.is_equal,
                    )
                inst.then_inc(vsem, 1)

        @block.tensor
        def _(t):
            if not do_pe:
                return
            prev_call = -1
            for ti in range(TT):
                if ti % CHT == 0:
                    t.wait_ge(vsem, ti // CHT + 1)
                ci = int(call_of_tile[ti])
                if ci != prev_call:
                    if nq == 1:
                        t.wait_ge(gsemA, 16 * (ci + 1))
                    else:
                        t.wait_ge(gsemA if ci % 2 == 0 else gsemB,
                                  16 * (ci // 2 + 1))
                    prev_call = ci
                o = int(tile_otile[ti])
                if do_out and first[ti] and o >= NPSUM:
                    t.wait_ge(csem, o - NPSUM + 1)
                slot = o % NPSUM
                vc = ti // CHT
                spos = ((vc % 2) * CHT + (ti - CHT * vc)) * 128
                base = (ci % NBUF) * max_call_tiles * D
                moff = base + int(tile_in_call[ti]) * D
                t.matmul(
                    psum[:, slot * 512:slot * 512 + D],
                    s_sb[:, spos:spos + 128],
                    msg[:, moff:moff + D],
                    start=bool(first[ti]),
                    stop=bool(last[ti]),
                ).then_inc(mmsem, 1)

        @block.scalar
        def _(sc):
            if not do_out:
                return
            for o in range(OT):
                sc.wait_ge(mmsem, int(M_o[o]))
                slot = o % NPSUM
                sc.copy(
                    out_sb[:, o * D:(o + 1) * D],
                    psum[:, slot * 512:slot * 512 + D],
                ).then_inc(csem, 1)

    nc.compile()
    return nc


def run_spmd(nc, per_core, trace=False):
    from concourse.bass_utils import run_bass_kernel_spmd
    return run_bass_kernel_spmd(
        nc, per_core, core_ids=list(range(len(per_core))), trace=trace
    )


def kernel(x, edge_index, _trace=False, _return_results=False):
    x = np.asarray(x, dtype=np.float32)
    per_core, sched = host_prep(x, edge_index)
    nc = build_bass(sched)
    res = run_spmd(nc, per_core, trace=_trace)
    outs = []
    for k in range(NCORES):
        o = np.asarray(res.results[k]["out"])          # [128, OT*D]
        o = o.reshape(128, OT, D).transpose(1, 0, 2).reshape(OUT_ROWS, D)
        outs.append(o[:SHARD])
    out = np.concatenate(outs, axis=0)
    if _return_results:
        return out, res
    return out
